# revision 1
# baseline (speedup 1.0000x reference)
"""Single-head causal attention (B=8, T=2048, C=1024, head_dim=64) on 8 TRN2 NeuronCores.

Sharding: data-parallel over batch -- one batch element per core, qkv weights
replicated. Host prep per core: x[b] is transposed to [C, T] and cast to fp16
(PE streams fp16 at 1 cycle/row vs 4 for fp32; fp16's 11-bit mantissa keeps the
end-to-end error ~1e-3, and all PSUM accumulation stays fp32). W is pre-packed
on host into the SBUF chunk layout so its DMA is one contiguous transfer; the
tiny constant matrices (causal mask, row-extract selector, identity) also come
from host so no GPSIMD library load lands in the critical preamble.

Device schedule:
  kqT  = Wkq^T x^T + b_kq     [128, T]  (k rows 0:64, q rows 64:128; biases via
                                         an augmented ones-row K=1 chunk)
  qT   = rows 64:128 of kqT moved to base partition 0 via a PE row-extract
  vT   = Wv^T x^T + b_v       [128, T] (rows 64:128 zero) -> v1 [s, 65] tiles
                                        via PE transpose (+ones denom column)
  per s-chunk j:  ST_j = K_j Q^T  [128 s, t], t >= 128j only (causal),
                  P^T = exp(0.125*ST) in up-to-1024-col chunks, diagonal block
                  masked upper-tri, then  out_acc[i] += P^T_j[:, i]^T [v|1]_j
                  for every 128-row t-tile i >= j (output lands in NORMAL
                  [t, 65] orientation; 4 accumulators packed per PSUM bank)
  after j = 4m+3: t-tiles 4m..4m+3 are complete -> divide by the denominator
                  column and DMA out. No output transposes needed anywhere.
"""

import numpy as np

import concourse.bass as bass
import concourse.mybir as mybir
from concourse import bacc
from concourse.bass import ts
from concourse.bass_utils import run_bass_kernel_spmd
from concourse.tile import TileContext

B, T, C = 8, 2048, 1024
HD = 64
N_CORES = 8
NJ = C // 128  # contraction chunks for the qkv projection
NT = T // 128  # 128-row tiles along T
NG = T // 512  # 512-col groups along T
FP16 = mybir.dt.float16
IDENT = mybir.ActivationFunctionType.Identity
CST_W = 8 * 192 + 2 + 128 + 64 + 128 + 128  # 1986
F32 = mybir.dt.float32
EXP = mybir.ActivationFunctionType.Exp


def build_nc() -> bass.Bass:
    nc = bacc.Bacc(None, target_bir_lowering=False)
    # w is pre-packed on host: [128, NJ*192] with w[p, j*192+m] = W[j*128+p, m]
    xt = nc.declare_dram_parameter("xt", [C, T], FP16, isOutput=False)
    # cst packs, per partition: NJ*192 w-chunk cols | bkq | bv | msk | sel | idh
    cst = nc.declare_dram_parameter("cst", [128, CST_W], FP16, isOutput=False)
    out = nc.declare_dram_parameter("out", [T, HD], F32, isOutput=True)

    with TileContext(nc) as tc:
        with (
            tc.tile_pool(name="consts", bufs=1) as consts,
            tc.tile_pool(name="xtp", bufs=NJ) as xtp,
            tc.tile_pool(name="kqv", bufs=1) as kqv,
            tc.tile_pool(name="ptp", bufs=3) as ptp,
            tc.tile_pool(name="epi", bufs=4) as epi,
        ):
            # --- constants: one contiguous DMA ---
            cst_sb = consts.tile([128, CST_W], FP16)
            w_sb = cst_sb  # cols j*192 + [0:128) = Wkq_j, + [128:192) = Wv_j
            bkq_sb = cst_sb[:, 1536:1537]
            bv_sb = cst_sb[0:64, 1537:1538]
            msk_sb = cst_sb[:, 1538:1666]
            sel_sb = cst_sb[:, 1666:1730]
            idh_sb = cst_sb[:, 1730:1858]
            sel2_sb = cst_sb[:, 1858:1986]
            wu_sb = consts.tile([1, 512], FP16)
            nc.vector.memset(wu_sb[:], 1.0)
            bias32 = consts.tile([128, 2], F32)

            # --- load x^T in 128-partition chunks ---
            xts = []
            for j in range(NJ):
                xt_t = xtp.tile([128, T], FP16, tag="xt")
                eng = nc.sync if j % 2 == 0 else nc.scalar
                eng.dma_start(out=xt_t[:], in_=xt[ts(j, 128), :])
                xts.append(xt_t)
                if j == 1:
                    nc.scalar.dma_start(out=cst_sb[:], in_=cst[:, :])
                    nc.vector.tensor_copy(bias32[:, 0:1], cst_sb[:, 1536:1537])
                    nc.vector.tensor_copy(bias32[0:64, 1:2], cst_sb[0:64, 1537:1538])

            # --- qkv projection: all four 512-col groups accumulate at once
            # (j-outer, paced by the xt chunk DMAs); PE warms up on dummy
            # matmuls while the first chunks stream in ---
            kqT = kqv.tile([128, T], FP16)
            vT = kqv.tile([128, T], FP16)  # rows 64:128 zero-padded for transpose
            qT = kqv.tile([64, T], FP16)
            v1 = kqv.tile([128, NT, 80], FP16)  # [s, hd | ones | pad] per t-tile
            nc.vector.memset(vT[64:128, :], 0.0)
            with tc.tile_pool(name="psp", bufs=8, space=bass.MemorySpace.PSUM) as psp:
                wu_ps = psp.tile([128, 512], F32, tag="p")
                for r in range(6):
                    nc.tensor.matmul(wu_ps[:], wu_sb[:, 0:128], wu_sb[:], start=True, stop=True)
                kq_accs = [psp.tile([128, 512], F32, tag="p", name=f"kq_acc{n}") for n in range(NG)]
                v_accs = [psp.tile([64, 512], F32, tag="p", name=f"v_acc{n}") for n in range(NG)]
                for j in range(NJ):
                    first, last = j == 0, j == NJ - 1
                    for n in range(NG):
                        nc.tensor.matmul(
                            kq_accs[n][:], w_sb[:, j * 192:j * 192 + 128], xts[j][:, ts(n, 512)],
                            start=first, stop=last,
                        )
                    for n in range(NG):
                        nc.tensor.matmul(
                            v_accs[n][:], w_sb[:, j * 192 + 128:j * 192 + 192], xts[j][:, ts(n, 512)],
                            start=first, stop=last,
                        )
                for r in range(3):
                    nc.tensor.matmul(wu_ps[:], wu_sb[:, 0:128], wu_sb[:], start=True, stop=True)
                # boundary, per 512-col group: bias copies -> q shift -> v1 tiles
                for n in range(NG):
                    nc.vector.tensor_scalar_add(kqT[:, ts(n, 512)], kq_accs[n][:], bias32[:, 0:1])
                    nc.sync.dma_start(out=qT[:, ts(n, 512)], in_=kqT[64:128, ts(n, 512)])
                    nc.tensor.matmul(wu_ps[:], wu_sb[:, 0:128], wu_sb[:], start=True, stop=True)
                    nc.vector.tensor_scalar_add(vT[0:64, ts(n, 512)], v_accs[n][:], bias32[0:64, 1:2])
                    for i in range(4 * n, 4 * n + 4):
                        tpv = psp.tile([128, 128], FP16, tag="p", name=f"tpv{i}")
                        nc.tensor.transpose(tpv[:], vT[:, ts(i, 128)], idh_sb)
                        nc.vector.tensor_copy(v1[:, i, 0:HD], tpv[:, 0:HD])
                        nc.vector.memset(v1[:, i, HD:HD + 1], 1.0)
                    nc.tensor.matmul(wu_ps[:], wu_sb[:, 0:128], wu_sb[:], start=True, stop=True)

            # --- attention, t-group outer: one outT accumulator live at a time,
            # ST pieces for two s-chunks share a [128,1024] PSUM tile and one exp ---
            with (
                tc.tile_pool(name="pso", bufs=2, space=bass.MemorySpace.PSUM) as pso,
                tc.tile_pool(name="pst", bufs=3, space=bass.MemorySpace.PSUM) as pst,
            ):
                def do_pair(g, p, acc):
                    gb = 512 * g
                    jmax = 4 * g + 3
                    jA, jB = 2 * p, 2 * p + 1
                    aA, aB = max(128 * jA, gb), max(128 * jB, gb)
                    stp = pst.tile([128, 1024], F32, tag="st", name=f"stp_{g}_{p}")
                    ptt = ptp.tile([128, 1024], FP16, tag="pt", name=f"ptt_{g}_{p}")
                    for jj, a, col in ((jA, aA, 0), (jB, aB, 512)):
                        nc.tensor.matmul(
                            stp[:, col + a - gb:col + 512],
                            kqT[0:64, ts(jj, 128)], qT[:, a:gb + 512],
                            start=True, stop=True,
                        )
                    if jB >= 4 * g:
                        for jj, a, col in ((jA, aA, 0), (jB, aB, 512)):
                            nc.scalar.activation(
                                ptt[:, col + a - gb:col + 512],
                                stp[:, col + a - gb:col + 512], EXP, scale=0.125,
                            )
                    else:
                        nc.scalar.activation(ptt[:], stp[:], EXP, scale=0.125)
                    for jj, a, col in ((jA, aA, 0), (jB, aB, 512)):
                        if jj >= 4 * g:
                            nc.vector.tensor_mul(
                                ptt[:, col + a - gb:col + a - gb + 128],
                                ptt[:, col + a - gb:col + a - gb + 128], msk_sb,
                            )
                        nc.tensor.matmul(
                            acc[:, a - gb:512], v1[:, jj, 0:65],
                            ptt[:, col + a - gb:col + 512],
                            start=(jj == 0), stop=(jj == jmax),
                        )

                def epilogue(g, acc):
                    eo = epi.tile([128, 512], FP16, tag="eo", name=f"eo{g}")
                    nc.vector.memset(eo[64:128, :], 0.0)
                    nc.vector.tensor_copy(eo[0:65, :], acc[:])
                    for l in range(4):
                        i = 4 * g + l
                        tp = pso.tile([128, 128], FP16, tag="o", name=f"tp{i}")
                        nc.tensor.transpose(tp[:], eo[:, ts(l, 128)], idh_sb)
                        rcp = epi.tile([128, 1], F32, tag="rcp", name=f"rcp{i}")
                        nc.vector.reciprocal(rcp[:], tp[:, HD:HD + 1])
                        ob = epi.tile([128, HD], F32, tag="ob", name=f"ob{i}")
                        nc.vector.tensor_scalar_mul(ob[:], tp[:, 0:HD], rcp[:])
                        oeng = nc.sync if l % 2 == 0 else nc.scalar
                        oeng.dma_start(out=out[ts(i, 128), :], in_=ob[:])

                for g in range(NG):
                    acc = pso.tile([65, 512], F32, tag="o", name=f"outT_acc{g}")
                    for p in range(2 * g + 2):
                        do_pair(g, p, acc)
                    epilogue(g, acc)
    nc.compile()
    return nc


_NC_CACHE = None


def _get_nc() -> bass.Bass:
    global _NC_CACHE
    if _NC_CACHE is None:
        _NC_CACHE = build_nc()
    return _NC_CACHE


def make_in_maps(x: np.ndarray, W: np.ndarray, b: np.ndarray) -> list[dict]:
    cst = np.zeros((128, CST_W), dtype=np.float16)
    # w chunks: cst[p, j*192+m] = W[j*128+p, m]
    cst[:, :NJ * 3 * HD] = (
        W.astype(np.float16).reshape(NJ, 128, 3 * HD).transpose(1, 0, 2).reshape(128, NJ * 3 * HD)
    )
    cst[:, 1536] = b[0:128].astype(np.float16)
    cst[0:64, 1537] = b[128:192].astype(np.float16)
    cst[:, 1538:1666] = np.triu(np.ones((128, 128), dtype=np.float16))  # keep s <= t
    sel = np.zeros((128, 64), dtype=np.float16)
    sel[np.arange(64) + 64, np.arange(64)] = 1.0
    cst[:, 1666:1730] = sel
    cst[:, 1730:1858] = np.eye(128, dtype=np.float16)
    sel2 = np.zeros((128, 128), dtype=np.float16)
    sel2[np.arange(64), np.arange(64) + 64] = 1.0
    cst[:, 1858:1986] = sel2
    cst = np.ascontiguousarray(cst)
    in_maps = []
    for core in range(N_CORES):
        xtc = np.ascontiguousarray(x[core].astype(np.float16).T)
        in_maps.append({"xt": xtc, "cst": cst})
    return in_maps


def run(x, W, b, trace: bool = False):
    """Returns (output [B, T, HD] fp32, BassKernelResults)."""
    x, W, b = np.asarray(x), np.asarray(W), np.asarray(b)
    nc = _get_nc()
    res = run_bass_kernel_spmd(nc, make_in_maps(x, W, b), list(range(N_CORES)), trace=trace)
    out = np.stack([res.results[i]["out"] for i in range(N_CORES)], axis=0)
    return out.astype(np.float32), res


def kernel(x, W, b) -> np.ndarray:
    out, _ = run(x, W, b)
    return out

